# revision 16
# baseline (speedup 1.0000x reference)
"""AttentionWithRoPE distributed Trainium2 kernel (8 NeuronCores).

Sharding: DP(2 batches) x TP(4 heads per core). Core c handles batch
g=c//4 and heads 4r..4r+3 (r=c%4). Each core loads only ITS batch's
activations (4MB not 8MB) -> startup DMA halves vs pure head-TP.
The 4 heads run as two "pseudo-batches" (head pairs 01 / 23) through the
same two-phase pass structure the TP8 kernel used for its two batches,
keeping the [128,1024] exp shape that saturates ScalarE (the bottleneck:
128 exps x ~1.34us = ~171us; everything else hides under it).

Output projection: out rows for seq-block r of batch g = full 1024-hid
contraction -> needs all 16 heads' ctx for those 512 q rows. Exchange is
split in TWO 8-core AllToAlls so only half sits on the critical tail:
  a2a#A: head-pair-01 ctx slabs, issued after phase C, hides under the
         ~84us of phase D. Its 4 in-group slabs feed the A-half of the
         output projection (64 matmuls incl. zero-padded cross-group
         slots) run in phase-D PE slack, accumulated into po[] (fp32
         SBUF) with the bias folded in.
  a2a#B: head-pair-23 slabs after phase D - the only serial tail
         (~29us wire) - then 64 matmuls + DVE add with po[] + out DMA.
Cross-group a2a slots are duplicates (each pass writes slots qs and
qs+4), and cross-group wo blocks are host-zeroed, which keeps the SPMD
instruction stream identical on every core without indirect addressing.

Scheduling (from perfetto analysis of the TP8 baseline):
  - every granule in phases C/D depends only on data already resident
    (same-batch x), killing the 16us PE head-of-line stall the TP8
    version hit waiting for batch-1 x chunks mid-pass.
  - x streams as 8 fat [128,2048] chunks over sync/gpsimd/vector queues;
    wqkv (q|k|v-major) + compact cos/sin ([128,4096], batch/row dedup)
    on scalar queue; wo A/B (2MB each) load during phase C on sync /
    vector.
  - warmup matmul chain at phase-A start ramps the HAM PE clock before
    the first real projection (first matmuls otherwise run ~1.6x slow).
  - rope half-rotation = PE matmul against a 0/1 permutation matrix
    (DVE cannot shift partitions); sin-multiply reads the swap from
    PSUM. exp on ScalarE in [128,1024] ops over 2-bank psum tiles.
  - ctx^T via M=65 matmuls with a ones-column in V: row 64 gives the
    softmax denominator free. Normalization: DMA-reshape the rowsum to
    [128,4] so reciprocal runs 128 lanes wide, hop back, gpsimd
    partition-broadcast, one DVE multiply. Hop DMAs ride Sync.
  - PSUM: 2x 2-bank score slots + 4x 1-bank acc slots = 8 banks.
Bias folds (host): v-bias folds into the output bias exactly (softmax
rows sum to 1); q pre-scaled by 1/sqrt(64); full output bias applied
once in the po[] copy. Compute dtype bf16 (fp32 PSUM accumulation).
"""

import numpy as np

HID = 1024
S = 2048
SB = 2 * S       # two pseudo-batches (head pairs), seq-concatenated
NHEAD = 16
D = 64
HPC = 4          # heads per core
OSL = 128        # hidden slice per head-pair (2 * D)
RB = 512         # seq rows per core after exchange
NC = 8
NG = 4           # group size (cores per batch)
ROPE_BASE = 10000.0

_cached = None
_last_in_maps = None

N_DUMMY = 24    # keep-PE-warm matmuls spanning the tail AllToAll wait
N_WARM = 10     # clock-ramp matmuls at kernel start


def _build_nc():
    import concourse.bacc as bacc
    import concourse.mybir as mybir
    from concourse import tile

    f32 = mybir.dt.float32
    bf16 = mybir.dt.bfloat16
    AF = mybir.ActivationFunctionType

    nc = bacc.Bacc(None, target_bir_lowering=False)

    xT = nc.declare_dram_parameter("xT", [HID, S], bf16, isOutput=False)
    wqkvd = nc.declare_dram_parameter("wqkv", [128, 6 * HID], bf16,
                                      isOutput=False)
    woad = nc.declare_dram_parameter("woLA", [128, 8 * HID], bf16,
                                     isOutput=False)
    wobd = nc.declare_dram_parameter("woLB", [128, 8 * HID], bf16,
                                     isOutput=False)
    bqkd = nc.declare_dram_parameter("bqk", [128, 4], f32, isOutput=False)
    bod = nc.declare_dram_parameter("bo2", [128, 8], f32, isOutput=False)
    csd = nc.declare_dram_parameter("cs", [128, SB], bf16, isOutput=False)
    permd = nc.declare_dram_parameter("perm", [128, 128], bf16,
                                      isOutput=False)
    out_ext = nc.declare_dram_parameter("out", [HID, RB], bf16, isOutput=True)

    a2aA_in = nc.dram_tensor("a2aA_in", [NC, OSL, RB], bf16)
    a2aA_out = nc.dram_tensor("a2aA_out", [NC, OSL, RB], bf16)
    a2aB_in = nc.dram_tensor("a2aB_in", [NC, OSL, RB], bf16)
    a2aB_out = nc.dram_tensor("a2aB_out", [NC, OSL, RB], bf16)

    NHC = HID // 128  # 8 hidden chunks

    with tile.TileContext(nc) as tc:
        with (
            tc.tile_pool(name="persist", bufs=1) as pp,
            tc.tile_pool(name="xs", bufs=8) as xp,
            tc.tile_pool(name="work", bufs=2) as wp,
            tc.tile_pool(name="exp", bufs=2) as ep,
        ):
            # ---------- inputs: x + wqkv-q are the critical wires; they go
            # FIRST on every queue. Other consts follow; wo loads move into
            # phase C so their 4MB never contends with startup HBM. ------
            wqkv = pp.tile([128, 6 * HID], bf16, tag="wqkv", name="wqkv")
            # q block first (needed by first granules).
            nc.scalar.dma_start(out=wqkv[:, 0:2048], in_=wqkvd[:, 0:2048])

            # x: 8 fat [128,2048] chunks: sync c0-2,c6; gpsimd c3-5,c7.
            xq = [None] * NHC
            order = [(0, nc.sync), (3, nc.gpsimd), (1, nc.sync),
                     (4, nc.gpsimd), (2, nc.sync), (5, nc.gpsimd),
                     (6, nc.sync), (7, nc.gpsimd)]
            for c, eng in order:
                xb = xp.tile([128, 2048], bf16, tag="xb", bufs=8)
                eng.dma_start(out=xb[:, :],
                              in_=xT[128 * c:128 * (c + 1), :])
                xq[c] = xb

            cs = pp.tile([128, SB], bf16, tag="cs", name="cs")
            # cos|sin for sg0 first so rope can start early.
            nc.scalar.dma_start(out=cs[:, 0:512], in_=csd[:, 0:512])
            nc.scalar.dma_start(out=cs[:, S:S + 512], in_=csd[:, S:S + 512])
            bqk = pp.tile([128, 4], f32, tag="bqk", name="bqk")
            nc.scalar.dma_start(out=bqk[:, :], in_=bqkd[:, :])
            perm = pp.tile([128, 128], bf16, tag="perm", name="perm")
            nc.scalar.dma_start(out=perm[:, :], in_=permd[:, :])
            # k then v blocks of wqkv; remaining cos/sin; output bias.
            nc.scalar.dma_start(out=wqkv[:, 2048:4096], in_=wqkvd[:, 2048:4096])
            nc.scalar.dma_start(out=wqkv[:, 4096:6144], in_=wqkvd[:, 4096:6144])
            nc.scalar.dma_start(out=cs[:, 512:S], in_=csd[:, 512:S])
            nc.scalar.dma_start(out=cs[:, S + 512:SB], in_=csd[:, S + 512:SB])
            bo_sb = pp.tile([128, 8], f32, tag="bo", name="bo")
            nc.scalar.dma_start(out=bo_sb[:, :], in_=bod[:, :])

            def wsl(t, p, c):   # lhsT slice: proj t, head-pair p, chunk c
                lo = 2048 * t + 1024 * p + 128 * c
                return wqkv[:, lo:lo + 128]

            # PSUM pools (8 banks exactly):
            #  psA "spsbig": 2x [128,1024] (scores)          -> 4 banks
            #  psB "acc":    4x [128,512]  (proj/swap/ctx/po)-> 4 banks
            _cmA = tc.tile_pool(name="psA", bufs=2, space="PSUM")
            _cmB = tc.tile_pool(name="psB", bufs=4, space="PSUM")
            psA = _cmA.__enter__()
            psB = _cmB.__enter__()

            # ---- HAM clock-ramp warmup: small matmul chain on perm ----
            wps = psB.tile([128, 128], f32, tag="acc", padded_shape=[128, 512],
                           name="warmps")
            for i in range(N_WARM):
                nc.tensor.matmul(wps[:, :], lhsT=perm[:, :], rhs=perm[:, :],
                                 start=(i == 0), stop=(i == N_WARM - 1))
            wsb = wp.tile([128, 128], bf16, tag="warmsb")
            nc.vector.tensor_copy(wsb[:, :], wps[:, :])
            deadw = nc.dram_tensor("deadw", [128, 128], bf16)
            nc.sync.dma_start(out=deadw[:, :], in_=wsb[:, :])

            qr = pp.tile([128, SB], bf16, tag="qr", name="qr")
            kr = pp.tile([128, SB], bf16, tag="kr", name="kr")
            vt = pp.tile([128, 2 * SB // 128, D + 1], bf16, tag="vt",
                         name="vt")
            nc.gpsimd.memset(vt[:, :, D:D + 1], 1.0)
            ones1 = pp.tile([1, 64], f32, tag="ones1", name="ones1")
            nc.gpsimd.memset(ones1[:, :], 1.0)
            ctxh = [pp.tile([64, SB], bf16, tag=f"ctx{h}", name=f"ctx{h}")
                    for h in range(2)]
            po = [pp.tile([128, RB], f32, tag=f"po{ot}", name=f"po{ot}")
                  for ot in range(8)]
            cxA = [pp.tile([128, RB], bf16, tag=f"cxA{s}", name=f"cxA{s}")
                   for s in range(NC)]
            cxB = [pp.tile([128, RB], bf16, tag=f"cxB{s}", name=f"cxB{s}")
                   for s in range(NC)]

            # ---------- emission helpers (all emit small blocks) ----------
            def rope_half(sg, hb, dst):
                sl = slice(512 * sg, 512 * (sg + 1))
                c0 = 512 * (sg % 4)
                swp = psB.tile([128, 512], f32, tag="acc")
                nc.tensor.matmul(swp[:, :], lhsT=perm[:, :], rhs=hb[:, :],
                                 start=True, stop=True)
                t1 = wp.tile([128, 512], f32, tag="ropet1")
                nc.vector.tensor_mul(t1[:, :], hb[:, :], cs[:, c0:c0 + 512])
                t2 = wp.tile([128, 512], f32, tag="ropet2")
                nc.vector.tensor_mul(
                    t2[:, :], swp[:, :], cs[:, S + c0:S + c0 + 512])
                nc.vector.tensor_add(dst[:, sl], t1[:, :], t2[:, :])

            def granules_for(sg, xlo):
                """Projection work for one (head-pair, seq group) as lists
                of small closures slippable into per-key-step PE slack."""
                p = sg // 4
                st8 = {}

                def qk_gr(t, g):
                    def run():
                        if g == 0:
                            st8[t] = psB.tile([128, 512], f32, tag="acc",
                                              name=f"qkps{sg}_{t}")
                        ps = st8[t]
                        for c in (2 * g, 2 * g + 1):
                            nc.tensor.matmul(
                                ps[:, :], lhsT=wsl(t, p, c),
                                rhs=xq[c][:, xlo:xlo + 512],
                                start=(c == 0), stop=(c == NHC - 1))
                        if g == 3:
                            hb = wp.tile([128, 512], bf16,
                                         tag=("qh" if t == 0 else "kh"),
                                         bufs=2)
                            nc.vector.tensor_scalar(
                                hb[:, :], ps[:, :],
                                0.125 if t == 0 else 1.0,
                                bqk[:, 2 * t + p:2 * t + p + 1],
                                mybir.AluOpType.mult, mybir.AluOpType.add)
                            st8[f"hb{t}"] = hb
                    return run

                def rope_gr(t):
                    def run():
                        rope_half(sg, st8[f"hb{t}"], qr if t == 0 else kr)
                    return run

                def v_gr(j, g):
                    def run():
                        if g == 0:
                            st8[f"v{j}"] = psB.tile(
                                [128, OSL], f32, tag="acc",
                                padded_shape=[128, 512],
                                name=f"vps{sg}_{j}")
                        ps = st8[f"v{j}"]
                        x0 = xlo + 128 * j
                        for c in (2 * g, 2 * g + 1):
                            nc.tensor.matmul(
                                ps[:, :], lhsT=xq[c][:, x0:x0 + 128],
                                rhs=wsl(2, p, c),
                                start=(c == 0), stop=(c == NHC - 1))
                        if g == 3:
                            st = 4 * sg + j
                            for h in range(2):
                                nc.vector.tensor_copy(
                                    vt[:, 2 * st + h, 0:D],
                                    ps[:, 64 * h:64 * (h + 1)])
                    return run

                qg = [qk_gr(0, g) for g in range(4)] + [rope_gr(0)]
                kg = [qk_gr(1, g) for g in range(4)] + [rope_gr(1)]
                vg = [v_gr(j, g) for j in range(4) for g in range(4)]
                return qg, kg, vg

            def proj_sg(sg, xlo):
                qg, kg, vg = granules_for(sg, xlo)
                for f in qg + kg + vg:
                    f()

            def qk_rope_only(sg, xlo):
                qg, kg, _ = granules_for(sg, xlo)
                for f in qg + kg:
                    f()

            def attn_pass(p, qs, granules=()):
                gq = list(granules)
                q0 = S * p + 512 * qs
                a2a_in = a2aA_in if p == 0 else a2aB_in
                cpsA = psB.tile([128, 512], f32, tag="acc")
                cpsB = psB.tile([128, 512], f32, tag="acc")
                for ks in range(16):
                    k0 = S * p + 128 * ks
                    kb = 16 * p + ks
                    sps = psA.tile([128, 1024], f32, tag="spsbig")
                    nc.tensor.matmul(
                        sps[:, 0:512], lhsT=kr[0:64, k0:k0 + 128],
                        rhs=qr[0:64, q0:q0 + 512], start=True, stop=True)
                    nc.tensor.matmul(
                        sps[:, 512:1024], lhsT=kr[64:128, k0:k0 + 128],
                        rhs=qr[64:128, q0:q0 + 512], start=True, stop=True)
                    et = ep.tile([128, 1024], bf16, tag="expT", bufs=4)
                    nc.scalar.activation(et[:, :], sps[:, :], AF.Exp)
                    nc.tensor.matmul(
                        cpsA[0:D + 1, :], lhsT=vt[:, 2 * kb, :],
                        rhs=et[:, 0:512], start=(ks == 0), stop=(ks == 15))
                    nc.tensor.matmul(
                        cpsB[0:D + 1, :], lhsT=vt[:, 2 * kb + 1, :],
                        rhs=et[:, 512:1024],
                        start=(ks == 0), stop=(ks == 15))
                    for _ in range(2):
                        if gq:
                            f = gq.pop(0)
                            if f is not None:
                                f()
                while gq:
                    f = gq.pop(0)
                    if f is not None:
                        f()
                for h, cps in ((0, cpsA), (1, cpsB)):
                    # One [65,512] copy to SBUF releases the ctx psum slot
                    # immediately; row 64 is the softmax rowsum. Reciprocal
                    # via DMA-reshape to [128,4] so it runs 128 lanes wide;
                    # broadcast back over partitions with a K=1 PE matmul
                    # against a ones row (faster than gpsimd broadcast and
                    # saves the hop-back DMA target being on partition 0).
                    cs65 = ep.tile([65, 512], f32, tag="rec65", bufs=3)
                    nc.vector.tensor_copy(cs65[:, :], cps[0:D + 1, :])
                    rsP = ep.tile([128, 4], f32, tag="rsP")
                    nc.sync.dma_start(out=rsP[:, :], in_=cs65[64:65, :])
                    rPr = ep.tile([128, 4], f32, tag="rPr")
                    nc.vector.reciprocal(rPr[:, :], rsP[:, :])
                    rec0 = ep.tile([1, 512], f32, tag="rec0")
                    nc.sync.dma_start(out=rec0[:, :], in_=rPr[:, :])
                    rbp = psB.tile([64, 512], f32, tag="acc",
                                   padded_shape=[128, 512])
                    nc.tensor.matmul(rbp[:, :], lhsT=ones1[:, :],
                                     rhs=rec0[:, :], start=True, stop=True)
                    nc.vector.tensor_mul(
                        ctxh[h][:, q0:q0 + 512], cs65[0:64, :], rbp[:, :])
                    # slabs for BOTH same-rank slots (cross-group slot is a
                    # duplicate; keeps the SPMD stream core-independent).
                    for dup in range(2):
                        eng = nc.gpsimd if dup == 0 else nc.sync
                        eng.dma_start(
                            out=a2a_in[4 * dup + qs, 64 * h:64 * (h + 1), :],
                            in_=ctxh[h][:, q0:q0 + 512])

            # ---------- phase A: head-pair-01 projections + rope ----------
            for sg in range(3):
                proj_sg(sg, 512 * sg)
            qk_rope_only(3, 512 * 3)

            # tiny warmup collective: pre-arms ncfw so the real AllToAlls'
            # trigger-to-start latency is paid here, off the critical path
            warm_in = nc.dram_tensor("warm_in", [NC, 1, 64], bf16)
            warm_out = nc.dram_tensor("warm_out", [NC, 1, 64], bf16)
            nc.gpsimd.collective_compute(
                "AllToAll", mybir.AluOpType.bypass,
                replica_groups=[list(range(NC))],
                ins=[warm_in.ap().opt()],
                outs=[warm_out.ap().opt()])

            # wo halves load during phase C (2MB each): issued after pass
            # C0 so the wire never contends with startup-critical HBM.
            woA = pp.tile([128, 8 * HID], bf16, tag="woLA", name="woA")
            woB = pp.tile([128, 8 * HID], bf16, tag="woLB", name="woB")

            def wo_load():
                # both on sync: the ACT queue must stay exp-only here.
                nc.sync.dma_start(out=woA[:, :], in_=woad[:, :])
                nc.sync.dma_start(out=woB[:, :], in_=wobd[:, :])

            # ---------- phase C: head-pair-01 passes; head-pair-23
            # projections drain as micro-granules inside the passes ------
            _, _, vg3 = granules_for(3, 512 * 3)
            gparts = {sg: granules_for(sg, 512 * (sg % 4))
                      for sg in range(4, 8)}
            gqC = list(vg3)
            for sg in range(4, 8):
                gqC.extend(gparts[sg][1])          # k + rope
            gqC.append(wo_load)
            for sg in range(4, 8):
                gqC.extend(gparts[sg][2])          # v
            gqC.extend(gparts[4][0])               # q sg4 (pass D0)
            per_pass = (len(gqC) + 3) // 4
            for i in range(4):
                take = gqC[:per_pass]
                gqC = gqC[per_pass:]
                attn_pass(0, i, take)

            # a2a#A: head-pair-01 slabs; hides under phase D.
            nc.gpsimd.collective_compute(
                "AllToAll", mybir.AluOpType.bypass,
                replica_groups=[list(range(NC))],
                ins=[a2aA_in.ap().opt()],
                outs=[a2aA_out.ap().opt()])

            # ---------- phase D: head-pair-23 passes + A-half outproj ----
            def cxA_load():
                for s in range(NC):
                    eng = nc.sync if s % 2 == 0 else nc.gpsimd
                    eng.dma_start(out=cxA[s][:, :], in_=a2aA_out[s, :, :])

            def potA_parts(ot):
                """A-half outproj for one 128-row out block, split into
                four 2-matmul closures so each fits per-key-step slack."""
                st = {}

                def part(i):
                    def run():
                        if i == 0:
                            st["ps"] = psB.tile([128, 512], f32, tag="acc",
                                                name=f"potA{ot}")
                        ps = st["ps"]
                        for s in (2 * i, 2 * i + 1):
                            nc.tensor.matmul(
                                ps[:, :],
                                lhsT=woA[:, 1024 * s + 128 * ot:
                                         1024 * s + 128 * (ot + 1)],
                                rhs=cxA[s][:, :],
                                start=(s == 0), stop=(s == NC - 1))
                        if i == 3:
                            nc.vector.tensor_scalar(
                                po[ot][:, :], ps[:, :], 1.0,
                                bo_sb[:, ot:ot + 1],
                                mybir.AluOpType.mult, mybir.AluOpType.add)
                    return run
                return [part(i) for i in range(4)]

            gqD = []
            for sg in (5, 6, 7):
                gqD.extend(gparts[sg][0])          # q sg5-7
            attn_pass(1, 0, gqD[:10])              # q sg5, sg6
            attn_pass(1, 1, gqD[10:])              # q sg7
            # cxA loads mid-D2: by then a2a#A is long done, so the waiting
            # DMAs never head-of-line-block the norm queues. potA work is
            # split 2-matmul-fine across late D2 + D3.
            attn_pass(1, 2, [None] * 16 + [cxA_load]
                      + potA_parts(0) + potA_parts(1))
            attn_pass(1, 3, [p for ot in range(2, 8) for p in potA_parts(ot)])

            # a2a#B: head-pair-23 slabs; the serial tail.
            nc.gpsimd.collective_compute(
                "AllToAll", mybir.AluOpType.bypass,
                replica_groups=[list(range(NC))],
                ins=[a2aB_in.ap().opt()],
                outs=[a2aB_out.ap().opt()])

            _cmB.__exit__(None, None, None)
            _cmA.__exit__(None, None, None)
            _cmO = tc.tile_pool(name="psO", bufs=1, space="PSUM")
            psO = _cmO.__enter__()

            # 8 persistent accumulators; the B-half outproj runs s-outer so
            # matmuls start as soon as each received slab lands instead of
            # after all 8. Keep-warm dummies (spanning the AllToAll wait)
            # accumulate into ops[0]; the first real s==0 matmul's start
            # flag resets the bank, so the garbage never escapes.
            ops = [psO.tile([128, 512], f32, tag=f"ops{ot}", name=f"ops{ot}")
                   for ot in range(8)]
            dumsrc = pp.tile([128, 512], bf16, tag="dumsrc")
            nc.gpsimd.memset(dumsrc[:, :], 0.0)
            nc.vector.tensor_copy(
                dumsrc[0:64, :], ctxh[1][:, SB - 512:SB])
            for i in range(N_DUMMY):
                nc.tensor.matmul(
                    ops[0][:, :], lhsT=woB[:, 0:128], rhs=dumsrc[:, :],
                    start=True, stop=True)

            for s in range(NC):
                eng = nc.sync if s % 2 == 0 else nc.gpsimd
                eng.dma_start(out=cxB[s][:, :], in_=a2aB_out[s, :, :])
            for s in range(NC):
                for ot in range(8):
                    nc.tensor.matmul(
                        ops[ot][:, :],
                        lhsT=woB[:, 1024 * s + 128 * ot:
                                 1024 * s + 128 * (ot + 1)],
                        rhs=cxB[s][:, :],
                        start=(s == 0), stop=(s == NC - 1))
                if s == NC - 1:
                    for ot in range(8):
                        osb = ep.tile([128, RB], bf16, tag="osb", bufs=3)
                        nc.vector.tensor_add(osb[:, :], ops[ot][:, :],
                                             po[ot][:, :])
                        eng = nc.sync if ot % 2 == 0 else nc.gpsimd
                        eng.dma_start(
                            out=out_ext[128 * ot:128 * (ot + 1), :],
                            in_=osb[:, :])
            _cmO.__exit__(None, None, None)

    nc.finalize()
    return nc


def _host_tables():
    inv = 1.0 / (ROPE_BASE ** (np.arange(0, D, 2, dtype=np.float64) / D))
    pos = np.arange(S, dtype=np.float64)
    freqs = np.outer(pos, inv)                      # [S, 32]
    emb = np.concatenate([freqs, freqs], axis=-1)   # [S, 64]
    cosT = np.cos(emb).T.astype(np.float32)         # [64, S]
    sinT = np.sin(emb).T.astype(np.float32)
    sinS = np.concatenate([-sinT[:32], sinT[32:]], axis=0)
    cos2 = np.ascontiguousarray(np.tile(cosT, (2, 1)))   # [128, S]
    sin2 = np.ascontiguousarray(np.tile(sinS, (2, 1)))
    return cos2, sin2


def kernel(**inputs):
    import ml_dtypes
    from concourse.bass_utils import run_bass_kernel_spmd

    global _cached, _last_in_maps
    if _cached is None:
        _cached = _build_nc()
    nc = _cached

    bf = ml_dtypes.bfloat16
    hs = np.asarray(inputs["hidden_states"], dtype=np.float32)
    Wq = np.asarray(inputs["Wq"], dtype=np.float32)
    bq = np.asarray(inputs["bq"], dtype=np.float32)
    Wk = np.asarray(inputs["Wk"], dtype=np.float32)
    bk = np.asarray(inputs["bk"], dtype=np.float32)
    Wv = np.asarray(inputs["Wv"], dtype=np.float32)
    bv = np.asarray(inputs["bv"], dtype=np.float32)
    Wo = np.asarray(inputs["Wo"], dtype=np.float32)
    bo = np.asarray(inputs["bo"], dtype=np.float32)

    cos2, sin2 = _host_tables()
    cs = np.ascontiguousarray(
        np.concatenate([cos2, sin2], axis=1)).astype(bf)   # [128, 2S]
    bo2 = bo + bv @ Wo.T                                 # fold v-bias exactly
    bo2m = np.ascontiguousarray(bo2.reshape(8, 128).T)   # [128, 8]
    pidx = np.arange(128)
    pm = np.where(pidx % 64 < 32, pidx + 32, pidx - 32)
    permM = np.zeros((128, 128), dtype=np.float32)
    permM[pm, pidx] = 1.0                                # [k, m]: k==perm(m)
    permM = permM.astype(bf)

    xTb = [np.ascontiguousarray(hs[g].T).astype(bf) for g in range(2)]

    in_maps = []
    for c in range(NC):
        g, r = divmod(c, NG)
        # wqkv: t-major, then head-pair, then 128-col chunk.
        wq6 = np.empty((128, 6 * 1024), dtype=np.float32)
        for t, W in enumerate((Wq, Wk, Wv)):
            for p in range(2):
                rows = slice(256 * r + 128 * p, 256 * r + 128 * (p + 1))
                wt = W[rows, :].T.reshape(8, 128, 128)      # [c, hid, out]
                wq6[:, 2048 * t + 1024 * p:2048 * t + 1024 * (p + 1)] = (
                    wt.transpose(1, 0, 2).reshape(128, 1024))
        # bqk cols: [q-p0, q-p1, k-p0, k-p1]
        bqk4 = np.empty((128, 4), dtype=np.float32)
        for t, b in enumerate((bq * 0.125, bk)):
            for p in range(2):
                rows = slice(256 * r + 128 * p, 256 * r + 128 * (p + 1))
                bqk4[:, 2 * t + p] = b[rows]
        # wo halves: block s = Wo cols for src s's head pair (A=01, B=23),
        # zeroed when src s is in the other batch group.
        woLA = np.zeros((128, 8 * 1024), dtype=np.float32)
        woLB = np.zeros((128, 8 * 1024), dtype=np.float32)
        for s in range(NC):
            if s // NG != g:
                continue
            base = 256 * (s % NG)
            woLA[:, 1024 * s:1024 * (s + 1)] = Wo[:, base:base + 128].T
            woLB[:, 1024 * s:1024 * (s + 1)] = (
                Wo[:, base + 128:base + 256].T)
        in_maps.append({
            "xT": xTb[g],
            "wqkv": np.ascontiguousarray(wq6).astype(bf),
            "woLA": np.ascontiguousarray(woLA).astype(bf),
            "woLB": np.ascontiguousarray(woLB).astype(bf),
            "bqk": np.ascontiguousarray(bqk4),
            "bo2": bo2m,
            "cs": cs,
            "perm": permM,
        })

    _last_in_maps = in_maps
    res = run_bass_kernel_spmd(nc, in_maps, core_ids=list(range(NC)))
    out = np.empty((2, S, HID), dtype=np.float32)
    for c in range(NC):
        g, r = divmod(c, NG)
        out[g, RB * r:RB * (r + 1), :] = res.results[c]["out"].T.astype(np.float32)
    return out


# revision 23
# speedup vs baseline: 1.2574x; 1.2574x over previous
"""AttentionWithRoPE distributed Trainium2 kernel (8 NeuronCores).

Sharding: DP(2 batches) x TP(4 heads per core). Core c handles batch
g=c//4 and heads 4r..4r+3 (r=c%4). Each core loads only ITS batch's
activations (4MB not 8MB) -> startup DMA halves vs pure head-TP.
The 4 heads run as two "pseudo-batches" (head pairs 01 / 23) through the
same two-phase pass structure the TP8 kernel used for its two batches,
keeping the [128,1024] exp shape that saturates ScalarE (the bottleneck:
128 exps x ~1.34us = ~171us; everything else hides under it).

Output projection: out rows for seq-block r of batch g = full 1024-hid
contraction -> needs all 16 heads' ctx for those 512 q rows. Exchange is
split in TWO 8-core AllToAlls so only half sits on the critical tail:
  a2a#A: head-pair-01 ctx slabs, issued after phase C, hides under the
         ~84us of phase D. Its 4 in-group slabs feed the A-half of the
         output projection (64 matmuls incl. zero-padded cross-group
         slots) run in phase-D PE slack, accumulated into po[] (fp32
         SBUF) with the bias folded in.
  a2a#B: head-pair-23 slabs after phase D - the only serial tail
         (~29us wire) - then 64 matmuls + DVE add with po[] + out DMA.
Cross-group a2a slots are duplicates (each pass writes slots qs and
qs+4), and cross-group wo blocks are host-zeroed, which keeps the SPMD
instruction stream identical on every core without indirect addressing.

Scheduling (from perfetto analysis of the TP8 baseline):
  - every granule in phases C/D depends only on data already resident
    (same-batch x), killing the 16us PE head-of-line stall the TP8
    version hit waiting for batch-1 x chunks mid-pass.
  - x streams as 8 fat [128,2048] chunks over sync/gpsimd/vector queues;
    wqkv (q|k|v-major) + compact cos/sin ([128,4096], batch/row dedup)
    on scalar queue; wo A/B (2MB each) load during phase C on sync /
    vector.
  - warmup matmul chain at phase-A start ramps the HAM PE clock before
    the first real projection (first matmuls otherwise run ~1.6x slow).
  - rope half-rotation = PE matmul against a 0/1 permutation matrix
    (DVE cannot shift partitions); sin-multiply reads the swap from
    PSUM. exp on ScalarE in [128,1024] ops over 2-bank psum tiles.
  - ctx^T via M=65 matmuls with a ones-column in V: row 64 gives the
    softmax denominator free. Normalization: DMA-reshape the rowsum to
    [128,4] so reciprocal runs 128 lanes wide, hop back, gpsimd
    partition-broadcast, one DVE multiply. Hop DMAs ride Sync.
  - PSUM: 2x 2-bank score slots + 4x 1-bank acc slots = 8 banks.
Bias folds (host): v-bias folds into the output bias exactly (softmax
rows sum to 1); q pre-scaled by 1/sqrt(64); full output bias applied
once in the po[] copy. Compute dtype bf16 (fp32 PSUM accumulation).
"""

import numpy as np

HID = 1024
S = 2048
SB = 2 * S       # two pseudo-batches (head pairs), seq-concatenated
NHEAD = 16
D = 64
HPC = 4          # heads per core
OSL = 128        # hidden slice per head-pair (2 * D)
RB = 512         # seq rows per core after exchange
NC = 8
NG = 4           # group size (cores per batch)
ROPE_BASE = 10000.0

_cached = None
_last_in_maps = None

N_DUMMY = 40    # keep-PE-warm matmuls spanning the tail AllToAll wait
N_WARM = 10     # clock-ramp matmuls at kernel start


def _build_nc():
    import concourse.bacc as bacc
    import concourse.mybir as mybir
    from concourse import tile

    f32 = mybir.dt.float32
    bf16 = mybir.dt.bfloat16
    AF = mybir.ActivationFunctionType

    nc = bacc.Bacc(None, target_bir_lowering=False)

    xT = nc.declare_dram_parameter("xT", [HID, S], bf16, isOutput=False)
    wqkvd = nc.declare_dram_parameter("wqkv", [128, 6 * HID], bf16,
                                      isOutput=False)
    woad = nc.declare_dram_parameter("woLA", [128, 8 * HID], bf16,
                                     isOutput=False)
    wobd = nc.declare_dram_parameter("woLB", [128, 8 * HID], bf16,
                                     isOutput=False)
    bqkd = nc.declare_dram_parameter("bqk", [128, 4], f32, isOutput=False)
    bod = nc.declare_dram_parameter("bo2", [128, 8], f32, isOutput=False)
    csd = nc.declare_dram_parameter("cs", [128, SB], bf16, isOutput=False)
    permd = nc.declare_dram_parameter("perm", [128, 128], bf16,
                                      isOutput=False)
    out_ext = nc.declare_dram_parameter("out", [HID, RB], bf16, isOutput=True)

    a2aA_in = nc.dram_tensor("a2aA_in", [NC, OSL, RB], bf16)
    a2aA_out = nc.dram_tensor("a2aA_out", [NC, OSL, RB], bf16)
    a2aB_in = nc.dram_tensor("a2aB_in", [NC, OSL, RB], bf16)
    a2aB_out = nc.dram_tensor("a2aB_out", [NC, OSL, RB], bf16)

    NHC = HID // 128  # 8 hidden chunks

    with tile.TileContext(nc) as tc:
        with (
            tc.tile_pool(name="persist", bufs=1) as pp,
            tc.tile_pool(name="xs", bufs=8) as xp,
            tc.tile_pool(name="work", bufs=2) as wp,
            tc.tile_pool(name="exp", bufs=2) as ep,
        ):
            # ---------- inputs: x + wqkv-q are the critical wires; they go
            # FIRST on every queue. Other consts follow; wo loads move into
            # phase C so their 4MB never contends with startup HBM. ------
            wqkv = pp.tile([128, 6 * HID], bf16, tag="wqkv", name="wqkv")
            # q block first (needed by first granules).
            nc.scalar.dma_start(out=wqkv[:, 0:2048], in_=wqkvd[:, 0:2048])

            # x: 8 fat [128,2048] chunks: sync c0-2,c6; gpsimd c3-5,c7.
            xq = [None] * NHC
            order = [(0, nc.sync), (3, nc.gpsimd), (1, nc.sync),
                     (4, nc.gpsimd), (2, nc.sync), (5, nc.gpsimd),
                     (6, nc.sync), (7, nc.gpsimd)]
            for c, eng in order:
                xb = xp.tile([128, 2048], bf16, tag="xb", bufs=8)
                eng.dma_start(out=xb[:, :],
                              in_=xT[128 * c:128 * (c + 1), :])
                xq[c] = xb

            cs = pp.tile([128, SB], bf16, tag="cs", name="cs")
            # cos|sin for sg0 first so rope can start early.
            nc.scalar.dma_start(out=cs[:, 0:512], in_=csd[:, 0:512])
            nc.scalar.dma_start(out=cs[:, S:S + 512], in_=csd[:, S:S + 512])
            bqk = pp.tile([128, 4], f32, tag="bqk", name="bqk")
            nc.scalar.dma_start(out=bqk[:, :], in_=bqkd[:, :])
            perm = pp.tile([128, 128], bf16, tag="perm", name="perm")
            nc.scalar.dma_start(out=perm[:, :], in_=permd[:, :])
            # k then v blocks of wqkv; remaining cos/sin; output bias.
            nc.scalar.dma_start(out=wqkv[:, 2048:4096], in_=wqkvd[:, 2048:4096])
            nc.scalar.dma_start(out=wqkv[:, 4096:6144], in_=wqkvd[:, 4096:6144])
            nc.scalar.dma_start(out=cs[:, 512:S], in_=csd[:, 512:S])
            nc.scalar.dma_start(out=cs[:, S + 512:SB], in_=csd[:, S + 512:SB])
            bo_sb = pp.tile([128, 8], f32, tag="bo", name="bo")
            nc.scalar.dma_start(out=bo_sb[:, :], in_=bod[:, :])

            def wsl(t, p, c):   # lhsT slice: proj t, head-pair p, chunk c
                lo = 2048 * t + 1024 * p + 128 * c
                return wqkv[:, lo:lo + 128]

            # PSUM pools (8 banks exactly):
            #  psA "spsbig": 2x [128,1024] (scores)          -> 4 banks
            #  psB "acc":    4x [128,512]  (proj/swap/ctx/po)-> 4 banks
            _cmA = tc.tile_pool(name="psA", bufs=2, space="PSUM")
            _cmB = tc.tile_pool(name="psB", bufs=4, space="PSUM")
            psA = _cmA.__enter__()
            psB = _cmB.__enter__()

            # ---- HAM clock-ramp warmup: small matmul chain on perm ----
            wps = psB.tile([128, 128], f32, tag="acc", padded_shape=[128, 512],
                           name="warmps")
            for i in range(N_WARM):
                nc.tensor.matmul(wps[:, :], lhsT=perm[:, :], rhs=perm[:, :],
                                 start=(i == 0), stop=(i == N_WARM - 1))
            wsb = wp.tile([128, 128], bf16, tag="warmsb")
            nc.vector.tensor_copy(wsb[:, :], wps[:, :])
            deadw = nc.dram_tensor("deadw", [128, 128], bf16)
            nc.sync.dma_start(out=deadw[:, :], in_=wsb[:, :])

            qr = pp.tile([128, SB], bf16, tag="qr", name="qr")
            kr = pp.tile([128, SB], bf16, tag="kr", name="kr")
            vt = pp.tile([128, 2 * SB // 128, D + 1], bf16, tag="vt",
                         name="vt")
            nc.gpsimd.memset(vt[:, :, D:D + 1], 1.0)

            ctxh = [pp.tile([64, SB], bf16, tag=f"ctx{h}", name=f"ctx{h}")
                    for h in range(2)]
            cxA = [pp.tile([128, RB], bf16, tag=f"cxA{s}", name=f"cxA{s}")
                   for s in range(NC)]
            cxB = [pp.tile([128, RB], bf16, tag=f"cxB{s}", name=f"cxB{s}")
                   for s in range(NC)]

            # ---------- emission helpers (all emit small blocks) ----------
            def rope_half(sg, hb, dst):
                sl = slice(512 * sg, 512 * (sg + 1))
                c0 = 512 * (sg % 4)
                swp = psB.tile([128, 512], f32, tag="acc")
                nc.tensor.matmul(swp[:, :], lhsT=perm[:, :], rhs=hb[:, :],
                                 start=True, stop=True)
                t1 = wp.tile([128, 512], f32, tag="ropet1")
                nc.vector.tensor_mul(t1[:, :], hb[:, :], cs[:, c0:c0 + 512])
                t2 = wp.tile([128, 512], f32, tag="ropet2")
                nc.vector.tensor_mul(
                    t2[:, :], swp[:, :], cs[:, S + c0:S + c0 + 512])
                nc.vector.tensor_add(dst[:, sl], t1[:, :], t2[:, :])

            def granules_for(sg, xlo):
                """Projection work for one (head-pair, seq group) as lists
                of small closures slippable into per-key-step PE slack."""
                p = sg // 4
                st8 = {}

                def qk_gr(t, g):
                    def run():
                        if g == 0:
                            st8[t] = psB.tile([128, 512], f32, tag="acc",
                                              name=f"qkps{sg}_{t}")
                        ps = st8[t]
                        for c in (2 * g, 2 * g + 1):
                            nc.tensor.matmul(
                                ps[:, :], lhsT=wsl(t, p, c),
                                rhs=xq[c][:, xlo:xlo + 512],
                                start=(c == 0), stop=(c == NHC - 1))
                        if g == 3:
                            hb = wp.tile([128, 512], bf16,
                                         tag=("qh" if t == 0 else "kh"),
                                         bufs=2)
                            nc.vector.tensor_scalar(
                                hb[:, :], ps[:, :],
                                0.125 if t == 0 else 1.0,
                                bqk[:, 2 * t + p:2 * t + p + 1],
                                mybir.AluOpType.mult, mybir.AluOpType.add)
                            st8[f"hb{t}"] = hb
                    return run

                def rope_gr(t):
                    def run():
                        rope_half(sg, st8[f"hb{t}"], qr if t == 0 else kr)
                    return run

                def v_gr(j, g):
                    def run():
                        if g == 0:
                            st8[f"v{j}"] = psB.tile(
                                [128, OSL], f32, tag="acc",
                                padded_shape=[128, 512],
                                name=f"vps{sg}_{j}")
                        ps = st8[f"v{j}"]
                        x0 = xlo + 128 * j
                        for c in (2 * g, 2 * g + 1):
                            nc.tensor.matmul(
                                ps[:, :], lhsT=xq[c][:, x0:x0 + 128],
                                rhs=wsl(2, p, c),
                                start=(c == 0), stop=(c == NHC - 1))
                        if g == 3:
                            st = 4 * sg + j
                            for h in range(2):
                                nc.vector.tensor_copy(
                                    vt[:, 2 * st + h, 0:D],
                                    ps[:, 64 * h:64 * (h + 1)])
                    return run

                qg = [qk_gr(0, g) for g in range(4)] + [rope_gr(0)]
                kg = [qk_gr(1, g) for g in range(4)] + [rope_gr(1)]
                vg = [v_gr(j, g) for j in range(4) for g in range(4)]
                return qg, kg, vg

            def proj_sg(sg, xlo):
                qg, kg, vg = granules_for(sg, xlo)
                for f in qg + kg + vg:
                    f()

            def qk_rope_only(sg, xlo):
                qg, kg, _ = granules_for(sg, xlo)
                for f in qg + kg:
                    f()

            def attn_pass(p, qs, granules=()):
                gq = list(granules)
                q0 = S * p + 512 * qs
                a2a_in = a2aA_in if p == 0 else a2aB_in
                cpsA = psB.tile([128, 512], f32, tag="acc")
                cpsB = psB.tile([128, 512], f32, tag="acc")
                for ks in range(16):
                    k0 = S * p + 128 * ks
                    kb = 16 * p + ks
                    sps = psA.tile([128, 1024], f32, tag="spsbig")
                    nc.tensor.matmul(
                        sps[:, 0:512], lhsT=kr[0:64, k0:k0 + 128],
                        rhs=qr[0:64, q0:q0 + 512], start=True, stop=True)
                    nc.tensor.matmul(
                        sps[:, 512:1024], lhsT=kr[64:128, k0:k0 + 128],
                        rhs=qr[64:128, q0:q0 + 512], start=True, stop=True)
                    et = ep.tile([128, 1024], bf16, tag="expT", bufs=4)
                    nc.scalar.activation(et[:, :], sps[:, :], AF.Exp)
                    nc.tensor.matmul(
                        cpsA[0:D + 1, :], lhsT=vt[:, 2 * kb, :],
                        rhs=et[:, 0:512], start=(ks == 0), stop=(ks == 15))
                    nc.tensor.matmul(
                        cpsB[0:D + 1, :], lhsT=vt[:, 2 * kb + 1, :],
                        rhs=et[:, 512:1024],
                        start=(ks == 0), stop=(ks == 15))
                    for _ in range(2):
                        if gq:
                            f = gq.pop(0)
                            if f is not None:
                                f()
                while gq:
                    f = gq.pop(0)
                    if f is not None:
                        f()
                for h, cps in ((0, cpsA), (1, cpsB)):
                    # One [65,512] copy to SBUF releases the ctx psum slot
                    # immediately; row 64 is the softmax rowsum. Reciprocal
                    # via DMA-reshape to [128,4] so it runs 128 lanes wide.
                    cs65 = ep.tile([65, 512], f32, tag="rec65", bufs=3)
                    nc.vector.tensor_copy(cs65[:, :], cps[0:D + 1, :])
                    rsP = ep.tile([128, 4], f32, tag="rsP")
                    nc.sync.dma_start(out=rsP[:, :], in_=cs65[64:65, :])
                    rPr = ep.tile([128, 4], f32, tag="rPr")
                    nc.vector.reciprocal(rPr[:, :], rsP[:, :])
                    rec0 = ep.tile([1, 512], f32, tag="rec0")
                    nc.sync.dma_start(out=rec0[:, :], in_=rPr[:, :])
                    rb = ep.tile([64, 512], f32, tag="recb")
                    nc.gpsimd.partition_broadcast(rb[:, :], rec0[:, :])
                    nc.vector.tensor_mul(
                        ctxh[h][:, q0:q0 + 512], cs65[0:64, :], rb[:, :])
                    # slabs for BOTH same-rank slots (cross-group slot is a
                    # duplicate; keeps the SPMD stream core-independent).
                    for dup in range(2):
                        eng = nc.gpsimd if dup == 0 else nc.sync
                        eng.dma_start(
                            out=a2a_in[4 * dup + qs, 64 * h:64 * (h + 1), :],
                            in_=ctxh[h][:, q0:q0 + 512])

            # ---------- phase A: head-pair-01 projections + rope ----------
            for sg in range(3):
                proj_sg(sg, 512 * sg)
            qk_rope_only(3, 512 * 3)

            # tiny warmup collective: pre-arms ncfw so the real AllToAlls'
            # trigger-to-start latency is paid here, off the critical path
            warm_in = nc.dram_tensor("warm_in", [NC, 1, 64], bf16)
            warm_out = nc.dram_tensor("warm_out", [NC, 1, 64], bf16)
            nc.gpsimd.collective_compute(
                "AllToAll", mybir.AluOpType.bypass,
                replica_groups=[list(range(NC))],
                ins=[warm_in.ap().opt()],
                outs=[warm_out.ap().opt()])

            # wo halves load during phase C (2MB each): issued after pass
            # C0 so the wire never contends with startup-critical HBM.
            woA = pp.tile([128, 8 * HID], bf16, tag="woLA", name="woA")
            woB = pp.tile([128, 8 * HID], bf16, tag="woLB", name="woB")

            def wo_load():
                # SCALAR engine ring: its DMA ring is empty during phase C,
                # so the 4MB wire never queues in front of the norm hop
                # DMAs (same-ring transfers execute in order). Costs two
                # ~0.8us pauses of the exp stream, once.
                nc.scalar.dma_start(out=woA[:, :], in_=woad[:, :])
                nc.scalar.dma_start(out=woB[:, :], in_=wobd[:, :])

            # ---------- phase C: head-pair-01 passes; head-pair-23
            # projections drain as micro-granules inside the passes ------
            _, _, vg3 = granules_for(3, 512 * 3)
            gparts = {sg: granules_for(sg, 512 * (sg % 4))
                      for sg in range(4, 8)}
            gqC = list(vg3)
            for sg in range(4, 8):
                gqC.extend(gparts[sg][1])          # k + rope
            gqC.append(wo_load)
            for sg in range(4, 8):
                gqC.extend(gparts[sg][2])          # v
            gqC.extend(gparts[4][0])               # q sg4 (pass D0)
            per_pass = (len(gqC) + 3) // 4
            for i in range(4):
                take = gqC[:per_pass]
                gqC = gqC[per_pass:]
                attn_pass(0, i, take)

            # a2a#A: head-pair-01 slabs; hides under phase D.
            nc.gpsimd.collective_compute(
                "AllToAll", mybir.AluOpType.bypass,
                replica_groups=[list(range(NC))],
                ins=[a2aA_in.ap().opt()],
                outs=[a2aA_out.ap().opt()])

            # ---------- phase D: head-pair-23 passes + A-half outproj ----
            def cxA_load():
                for s in range(NC):
                    eng = nc.sync if s % 2 == 0 else nc.gpsimd
                    eng.dma_start(out=cxA[s][:, :], in_=a2aA_out[s, :, :])

            gqD = []
            for sg in (5, 6, 7):
                gqD.extend(gparts[sg][0])          # q sg5-7
            attn_pass(1, 0, gqD[:10])              # q sg5, sg6
            attn_pass(1, 1, gqD[10:])              # q sg7
            # cxA loads mid-D2: by then a2a#A is long done, so the waiting
            # DMAs never head-of-line-block the norm queues.
            attn_pass(1, 2, [None] * 16 + [cxA_load])
            attn_pass(1, 3)

            # a2a#B: head-pair-23 slabs; the serial tail.
            nc.gpsimd.collective_compute(
                "AllToAll", mybir.AluOpType.bypass,
                replica_groups=[list(range(NC))],
                ins=[a2aB_in.ap().opt()],
                outs=[a2aB_out.ap().opt()])

            _cmB.__exit__(None, None, None)
            _cmA.__exit__(None, None, None)
            _cmO = tc.tile_pool(name="psO", bufs=1, space="PSUM")
            psO = _cmO.__enter__()

            # 8 persistent accumulators. The A-half outproj (deps all ready
            # at D3 end) runs INSIDE the a2a#B wait and doubles as PE
            # keep-warm; its banks stay open until the B-half matmuls
            # continue the accumulation once the exchange lands. Keep-warm
            # dummies before it accumulate garbage into ops[0]; the A-half
            # s==0 matmul's start flag resets the bank.
            ops = [psO.tile([128, 512], f32, tag=f"ops{ot}", name=f"ops{ot}")
                   for ot in range(8)]
            dumsrc = pp.tile([128, 512], bf16, tag="dumsrc")
            nc.gpsimd.memset(dumsrc[:, :], 0.0)
            nc.vector.tensor_copy(
                dumsrc[0:64, :], ctxh[1][:, SB - 512:SB])
            for i in range(N_DUMMY):
                nc.tensor.matmul(
                    ops[0][:, :], lhsT=woB[:, 0:128], rhs=dumsrc[:, :],
                    start=True, stop=True)
            for s in range(NC):
                for ot in range(8):
                    nc.tensor.matmul(
                        ops[ot][:, :],
                        lhsT=woA[:, 1024 * s + 128 * ot:
                                 1024 * s + 128 * (ot + 1)],
                        rhs=cxA[s][:, :],
                        start=(s == 0), stop=False)

            for s in range(NC):
                eng = nc.sync if s % 2 == 0 else nc.gpsimd
                eng.dma_start(out=cxB[s][:, :], in_=a2aB_out[s, :, :])
            for s in range(NC):
                for ot in range(8):
                    nc.tensor.matmul(
                        ops[ot][:, :],
                        lhsT=woB[:, 1024 * s + 128 * ot:
                                 1024 * s + 128 * (ot + 1)],
                        rhs=cxB[s][:, :],
                        start=False, stop=(s == NC - 1))
                if s == NC - 1:
                    for ot in range(8):
                        osb = ep.tile([128, RB], bf16, tag="osb", bufs=3)
                        nc.vector.tensor_scalar(
                            osb[:, :], ops[ot][:, :], 1.0,
                            bo_sb[:, ot:ot + 1],
                            mybir.AluOpType.mult, mybir.AluOpType.add)
                        eng = nc.sync if ot % 2 == 0 else nc.gpsimd
                        eng.dma_start(
                            out=out_ext[128 * ot:128 * (ot + 1), :],
                            in_=osb[:, :])
            _cmO.__exit__(None, None, None)

    nc.finalize()
    return nc


def _host_tables():
    inv = 1.0 / (ROPE_BASE ** (np.arange(0, D, 2, dtype=np.float64) / D))
    pos = np.arange(S, dtype=np.float64)
    freqs = np.outer(pos, inv)                      # [S, 32]
    emb = np.concatenate([freqs, freqs], axis=-1)   # [S, 64]
    cosT = np.cos(emb).T.astype(np.float32)         # [64, S]
    sinT = np.sin(emb).T.astype(np.float32)
    sinS = np.concatenate([-sinT[:32], sinT[32:]], axis=0)
    cos2 = np.ascontiguousarray(np.tile(cosT, (2, 1)))   # [128, S]
    sin2 = np.ascontiguousarray(np.tile(sinS, (2, 1)))
    return cos2, sin2


def kernel(**inputs):
    import ml_dtypes
    from concourse.bass_utils import run_bass_kernel_spmd

    global _cached, _last_in_maps
    if _cached is None:
        _cached = _build_nc()
    nc = _cached

    bf = ml_dtypes.bfloat16
    hs = np.asarray(inputs["hidden_states"], dtype=np.float32)
    Wq = np.asarray(inputs["Wq"], dtype=np.float32)
    bq = np.asarray(inputs["bq"], dtype=np.float32)
    Wk = np.asarray(inputs["Wk"], dtype=np.float32)
    bk = np.asarray(inputs["bk"], dtype=np.float32)
    Wv = np.asarray(inputs["Wv"], dtype=np.float32)
    bv = np.asarray(inputs["bv"], dtype=np.float32)
    Wo = np.asarray(inputs["Wo"], dtype=np.float32)
    bo = np.asarray(inputs["bo"], dtype=np.float32)

    cos2, sin2 = _host_tables()
    cs = np.ascontiguousarray(
        np.concatenate([cos2, sin2], axis=1)).astype(bf)   # [128, 2S]
    bo2 = bo + bv @ Wo.T                                 # fold v-bias exactly
    bo2m = np.ascontiguousarray(bo2.reshape(8, 128).T)   # [128, 8]
    pidx = np.arange(128)
    pm = np.where(pidx % 64 < 32, pidx + 32, pidx - 32)
    permM = np.zeros((128, 128), dtype=np.float32)
    permM[pm, pidx] = 1.0                                # [k, m]: k==perm(m)
    permM = permM.astype(bf)

    xTb = [np.ascontiguousarray(hs[g].T).astype(bf) for g in range(2)]

    in_maps = []
    for c in range(NC):
        g, r = divmod(c, NG)
        # wqkv: t-major, then head-pair, then 128-col chunk.
        wq6 = np.empty((128, 6 * 1024), dtype=np.float32)
        for t, W in enumerate((Wq, Wk, Wv)):
            for p in range(2):
                rows = slice(256 * r + 128 * p, 256 * r + 128 * (p + 1))
                wt = W[rows, :].T.reshape(8, 128, 128)      # [c, hid, out]
                wq6[:, 2048 * t + 1024 * p:2048 * t + 1024 * (p + 1)] = (
                    wt.transpose(1, 0, 2).reshape(128, 1024))
        # bqk cols: [q-p0, q-p1, k-p0, k-p1]
        bqk4 = np.empty((128, 4), dtype=np.float32)
        for t, b in enumerate((bq * 0.125, bk)):
            for p in range(2):
                rows = slice(256 * r + 128 * p, 256 * r + 128 * (p + 1))
                bqk4[:, 2 * t + p] = b[rows]
        # wo halves: block s = Wo cols for src s's head pair (A=01, B=23),
        # zeroed when src s is in the other batch group.
        woLA = np.zeros((128, 8 * 1024), dtype=np.float32)
        woLB = np.zeros((128, 8 * 1024), dtype=np.float32)
        for s in range(NC):
            if s // NG != g:
                continue
            base = 256 * (s % NG)
            woLA[:, 1024 * s:1024 * (s + 1)] = Wo[:, base:base + 128].T
            woLB[:, 1024 * s:1024 * (s + 1)] = (
                Wo[:, base + 128:base + 256].T)
        in_maps.append({
            "xT": xTb[g],
            "wqkv": np.ascontiguousarray(wq6).astype(bf),
            "woLA": np.ascontiguousarray(woLA).astype(bf),
            "woLB": np.ascontiguousarray(woLB).astype(bf),
            "bqk": np.ascontiguousarray(bqk4),
            "bo2": bo2m,
            "cs": cs,
            "perm": permM,
        })

    _last_in_maps = in_maps
    res = run_bass_kernel_spmd(nc, in_maps, core_ids=list(range(NC)))
    out = np.empty((2, S, HID), dtype=np.float32)
    for c in range(NC):
        g, r = divmod(c, NG)
        out[g, RB * r:RB * (r + 1), :] = res.results[c]["out"].T.astype(np.float32)
    return out


# revision 32
# speedup vs baseline: 1.3118x; 1.0433x over previous
"""AttentionWithRoPE distributed Trainium2 kernel (8 NeuronCores).

Sharding: DP(2 batches) x TP(4 heads per core). Core c handles batch
g=c//4 and heads 4r..4r+3 (r=c%4). Each core loads only ITS batch's
activations (4MB not 8MB) -> startup DMA halves vs pure head-TP.
The 4 heads run as two "pseudo-batches" (head pairs 01 / 23) through the
same two-phase pass structure the TP8 kernel used for its two batches,
keeping the [128,1024] exp shape that saturates ScalarE (the bottleneck:
128 exps x ~1.34us = ~171us; everything else hides under it).

Output projection: out rows for seq-block r of batch g = full 1024-hid
contraction -> needs all 16 heads' ctx for those 512 q rows. Exchange is
split in TWO 8-core AllToAlls so only half sits on the critical tail:
  a2a#A: head-pair-01 ctx slabs, issued after phase C, hides under the
         ~84us of phase D. Its 4 in-group slabs feed the A-half of the
         output projection (64 matmuls incl. zero-padded cross-group
         slots) run in phase-D PE slack, accumulated into po[] (fp32
         SBUF) with the bias folded in.
  a2a#B: head-pair-23 slabs after phase D - the only serial tail
         (~29us wire) - then 64 matmuls + DVE add with po[] + out DMA.
Cross-group a2a slots are duplicates (each pass writes slots qs and
qs+4), and cross-group wo blocks are host-zeroed, which keeps the SPMD
instruction stream identical on every core without indirect addressing.

Scheduling (from perfetto analysis of the TP8 baseline):
  - every granule in phases C/D depends only on data already resident
    (same-batch x), killing the 16us PE head-of-line stall the TP8
    version hit waiting for batch-1 x chunks mid-pass.
  - x streams as 8 fat [128,2048] chunks over sync/gpsimd/vector queues;
    wqkv (q|k|v-major) + compact cos/sin ([128,4096], batch/row dedup)
    on scalar queue; wo A/B (2MB each) load during phase C on sync /
    vector.
  - warmup matmul chain at phase-A start ramps the HAM PE clock before
    the first real projection (first matmuls otherwise run ~1.6x slow).
  - rope half-rotation = PE matmul against a 0/1 permutation matrix
    (DVE cannot shift partitions); sin-multiply reads the swap from
    PSUM. exp on ScalarE in [128,1024] ops over 2-bank psum tiles.
  - ctx^T via M=65 matmuls with a ones-column in V: row 64 gives the
    softmax denominator free. Normalization: DMA-reshape the rowsum to
    [128,4] so reciprocal runs 128 lanes wide, hop back, gpsimd
    partition-broadcast, one DVE multiply. Hop DMAs ride Sync.
  - PSUM: 2x 2-bank score slots + 4x 1-bank acc slots = 8 banks.
Bias folds (host): v-bias folds into the output bias exactly (softmax
rows sum to 1); q pre-scaled by 1/sqrt(64); full output bias applied
once in the po[] copy. Compute dtype bf16 (fp32 PSUM accumulation).
"""

import numpy as np

HID = 1024
S = 2048
SB = 2 * S       # two pseudo-batches (head pairs), seq-concatenated
NHEAD = 16
D = 64
HPC = 4          # heads per core
OSL = 128        # hidden slice per head-pair (2 * D)
RB = 512         # seq rows per core after exchange
NC = 8
NG = 4           # group size (cores per batch)
ROPE_BASE = 10000.0

_cached = None
_last_in_maps = None

N_DUMMY = 40    # keep-PE-warm matmuls spanning the tail AllToAll wait
N_WARM = 10     # clock-ramp matmuls at kernel start


def _build_nc():
    import concourse.bacc as bacc
    import concourse.mybir as mybir
    from concourse import tile

    f32 = mybir.dt.float32
    bf16 = mybir.dt.bfloat16
    AF = mybir.ActivationFunctionType

    nc = bacc.Bacc(None, target_bir_lowering=False)

    xT = nc.declare_dram_parameter("xT", [HID, S], bf16, isOutput=False)
    wqkvd = nc.declare_dram_parameter("wqkv", [128, 6 * HID], bf16,
                                      isOutput=False)
    woad = nc.declare_dram_parameter("woLA", [128, 4 * HID], bf16,
                                     isOutput=False)
    wobd = nc.declare_dram_parameter("woLB", [128, 4 * HID], bf16,
                                     isOutput=False)
    bqkd = nc.declare_dram_parameter("bqk", [128, 4], f32, isOutput=False)
    bod = nc.declare_dram_parameter("bo2", [128, 8], f32, isOutput=False)
    csd = nc.declare_dram_parameter("cs", [128, SB], bf16, isOutput=False)
    permd = nc.declare_dram_parameter("perm", [128, 128], bf16,
                                      isOutput=False)
    out_ext = nc.declare_dram_parameter("out", [HID, RB], bf16, isOutput=True)

    # a2a tensors kept flat [NC*OSL, RB] so received slabs can be read
    # with a runtime (partition_id-derived) row offset.
    a2aA_in = nc.dram_tensor("a2aA_in", [NC * OSL, RB], bf16)
    a2aA_out = nc.dram_tensor("a2aA_out", [NC * OSL, RB], bf16)
    a2aB_in = nc.dram_tensor("a2aB_in", [NC * OSL, RB], bf16)
    a2aB_out = nc.dram_tensor("a2aB_out", [NC * OSL, RB], bf16)

    NHC = HID // 128  # 8 hidden chunks

    with tile.TileContext(nc) as tc:
        with (
            tc.tile_pool(name="persist", bufs=1) as pp,
            tc.tile_pool(name="xs", bufs=8) as xp,
            tc.tile_pool(name="work", bufs=2) as wp,
            tc.tile_pool(name="exp", bufs=2) as ep,
        ):
            # ---------- inputs: x + wqkv-q are the critical wires; they go
            # FIRST on every queue. Other consts follow; wo loads move into
            # phase C so their 4MB never contends with startup HBM. ------
            wqkv = pp.tile([128, 6 * HID], bf16, tag="wqkv", name="wqkv")
            # q block first (needed by first granules).
            nc.scalar.dma_start(out=wqkv[:, 0:2048], in_=wqkvd[:, 0:2048])

            # x: 8 fat [128,2048] chunks, even on sync / odd on gpsimd so
            # granule chunk-PAIRS (2g, 2g+1) land together.
            xq = [None] * NHC
            order = [(0, nc.sync), (1, nc.gpsimd), (2, nc.sync),
                     (3, nc.gpsimd), (4, nc.sync), (5, nc.gpsimd),
                     (6, nc.sync), (7, nc.gpsimd)]
            for c, eng in order:
                xb = xp.tile([128, 2048], bf16, tag="xb", bufs=8)
                eng.dma_start(out=xb[:, :],
                              in_=xT[128 * c:128 * (c + 1), :])
                xq[c] = xb

            cs = pp.tile([128, SB], bf16, tag="cs", name="cs")
            # cos|sin for sg0 first so rope can start early.
            nc.scalar.dma_start(out=cs[:, 0:512], in_=csd[:, 0:512])
            nc.scalar.dma_start(out=cs[:, S:S + 512], in_=csd[:, S:S + 512])
            bqk = pp.tile([128, 4], f32, tag="bqk", name="bqk")
            nc.scalar.dma_start(out=bqk[:, :], in_=bqkd[:, :])
            perm = pp.tile([128, 128], bf16, tag="perm", name="perm")
            nc.scalar.dma_start(out=perm[:, :], in_=permd[:, :])
            # k then v blocks of wqkv; remaining cos/sin; output bias.
            nc.scalar.dma_start(out=wqkv[:, 2048:4096], in_=wqkvd[:, 2048:4096])
            nc.scalar.dma_start(out=wqkv[:, 4096:6144], in_=wqkvd[:, 4096:6144])
            nc.scalar.dma_start(out=cs[:, 512:S], in_=csd[:, 512:S])
            nc.scalar.dma_start(out=cs[:, S + 512:SB], in_=csd[:, S + 512:SB])
            bo_sb = pp.tile([128, 8], f32, tag="bo", name="bo")
            nc.scalar.dma_start(out=bo_sb[:, :], in_=bod[:, :])

            def wsl(t, p, c):   # lhsT slice: proj t, head-pair p, chunk c
                lo = 2048 * t + 1024 * p + 128 * c
                return wqkv[:, lo:lo + 128]

            # PSUM pools (8 banks exactly):
            #  psA "spsbig": 2x [128,1024] (scores)          -> 4 banks
            #  psB "acc":    4x [128,512]  (proj/swap/ctx/po)-> 4 banks
            _cmA = tc.tile_pool(name="psA", bufs=2, space="PSUM")
            _cmB = tc.tile_pool(name="psB", bufs=4, space="PSUM")
            psA = _cmA.__enter__()
            psB = _cmB.__enter__()

            # ---- HAM clock-ramp warmup: small matmul chain on perm ----
            wps = psB.tile([128, 128], f32, tag="acc", padded_shape=[128, 512],
                           name="warmps")
            for i in range(N_WARM):
                nc.tensor.matmul(wps[:, :], lhsT=perm[:, :], rhs=perm[:, :],
                                 start=(i == 0), stop=(i == N_WARM - 1))
            wsb = wp.tile([128, 128], bf16, tag="warmsb")
            nc.vector.tensor_copy(wsb[:, :], wps[:, :])
            deadw = nc.dram_tensor("deadw", [128, 128], bf16)
            nc.sync.dma_start(out=deadw[:, :], in_=wsb[:, :])

            qr = pp.tile([128, SB], bf16, tag="qr", name="qr")
            kr = pp.tile([128, SB], bf16, tag="kr", name="kr")
            vt = pp.tile([128, 2 * SB // 128, D + 1], bf16, tag="vt",
                         name="vt")
            nc.gpsimd.memset(vt[:, :, D:D + 1], 1.0)

            ctxh = [pp.tile([64, SB], bf16, tag=f"ctx{h}", name=f"ctx{h}")
                    for h in range(2)]
            cxA = [pp.tile([128, RB], bf16, tag=f"cxA{s}", name=f"cxA{s}")
                   for s in range(NG)]
            cxB = [pp.tile([128, RB], bf16, tag=f"cxB{s}", name=f"cxB{s}")
                   for s in range(NG)]

            # ---------- emission helpers (all emit small blocks) ----------
            def rope_half(sg, hb, dst):
                sl = slice(512 * sg, 512 * (sg + 1))
                c0 = 512 * (sg % 4)
                swp = psB.tile([128, 512], f32, tag="acc")
                nc.tensor.matmul(swp[:, :], lhsT=perm[:, :], rhs=hb[:, :],
                                 start=True, stop=True)
                t1 = wp.tile([128, 512], f32, tag="ropet1")
                nc.vector.tensor_mul(t1[:, :], hb[:, :], cs[:, c0:c0 + 512])
                t2 = wp.tile([128, 512], f32, tag="ropet2")
                nc.vector.tensor_mul(
                    t2[:, :], swp[:, :], cs[:, S + c0:S + c0 + 512])
                nc.vector.tensor_add(dst[:, sl], t1[:, :], t2[:, :])

            def granules_for(sg, xlo):
                """Projection work for one (head-pair, seq group) as lists
                of small closures slippable into per-key-step PE slack."""
                p = sg // 4
                st8 = {}

                def qk_gr(t, g):
                    def run():
                        if g == 0:
                            st8[t] = psB.tile([128, 512], f32, tag="acc",
                                              name=f"qkps{sg}_{t}")
                        ps = st8[t]
                        for c in (2 * g, 2 * g + 1):
                            nc.tensor.matmul(
                                ps[:, :], lhsT=wsl(t, p, c),
                                rhs=xq[c][:, xlo:xlo + 512],
                                start=(c == 0), stop=(c == NHC - 1))
                        if g == 3:
                            hb = wp.tile([128, 512], bf16,
                                         tag=("qh" if t == 0 else "kh"),
                                         bufs=2)
                            nc.vector.tensor_scalar(
                                hb[:, :], ps[:, :],
                                0.125 if t == 0 else 1.0,
                                bqk[:, 2 * t + p:2 * t + p + 1],
                                mybir.AluOpType.mult, mybir.AluOpType.add)
                            st8[f"hb{t}"] = hb
                    return run

                def rope_gr(t):
                    def run():
                        rope_half(sg, st8[f"hb{t}"], qr if t == 0 else kr)
                    return run

                def v_gr(j, g):
                    def run():
                        if g == 0:
                            st8[f"v{j}"] = psB.tile(
                                [128, OSL], f32, tag="acc",
                                padded_shape=[128, 512],
                                name=f"vps{sg}_{j}")
                        ps = st8[f"v{j}"]
                        x0 = xlo + 128 * j
                        for c in (2 * g, 2 * g + 1):
                            nc.tensor.matmul(
                                ps[:, :], lhsT=xq[c][:, x0:x0 + 128],
                                rhs=wsl(2, p, c),
                                start=(c == 0), stop=(c == NHC - 1))
                        if g == 3:
                            st = 4 * sg + j
                            for h in range(2):
                                nc.vector.tensor_copy(
                                    vt[:, 2 * st + h, 0:D],
                                    ps[:, 64 * h:64 * (h + 1)])
                    return run

                qg = [qk_gr(0, g) for g in range(4)] + [rope_gr(0)]
                kg = [qk_gr(1, g) for g in range(4)] + [rope_gr(1)]
                vg = [v_gr(j, g) for j in range(4) for g in range(4)]
                return qg, kg, vg

            def proj_sg(sg, xlo):
                qg, kg, vg = granules_for(sg, xlo)
                for f in qg + kg + vg:
                    f()

            def qk_rope_only(sg, xlo):
                qg, kg, _ = granules_for(sg, xlo)
                for f in qg + kg:
                    f()

            def attn_pass(p, qs, granules=()):
                gq = list(granules)
                q0 = S * p + 512 * qs
                a2a_in = a2aA_in if p == 0 else a2aB_in
                cpsA = psB.tile([128, 512], f32, tag="acc")
                cpsB = psB.tile([128, 512], f32, tag="acc")
                for ks in range(16):
                    k0 = S * p + 128 * ks
                    kb = 16 * p + ks
                    sps = psA.tile([128, 1024], f32, tag="spsbig")
                    nc.tensor.matmul(
                        sps[:, 0:512], lhsT=kr[0:64, k0:k0 + 128],
                        rhs=qr[0:64, q0:q0 + 512], start=True, stop=True)
                    nc.tensor.matmul(
                        sps[:, 512:1024], lhsT=kr[64:128, k0:k0 + 128],
                        rhs=qr[64:128, q0:q0 + 512], start=True, stop=True)
                    et = ep.tile([128, 1024], bf16, tag="expT", bufs=4)
                    nc.scalar.activation(et[:, :], sps[:, :], AF.Exp)
                    nc.tensor.matmul(
                        cpsA[0:D + 1, :], lhsT=vt[:, 2 * kb, :],
                        rhs=et[:, 0:512], start=(ks == 0), stop=(ks == 15))
                    nc.tensor.matmul(
                        cpsB[0:D + 1, :], lhsT=vt[:, 2 * kb + 1, :],
                        rhs=et[:, 512:1024],
                        start=(ks == 0), stop=(ks == 15))
                    for _ in range(2):
                        if gq:
                            f = gq.pop(0)
                            if f is not None:
                                f()
                while gq:
                    f = gq.pop(0)
                    if f is not None:
                        f()
                for h, cps in ((0, cpsA), (1, cpsB)):
                    # One [65,512] copy to SBUF releases the ctx psum slot
                    # immediately; row 64 is the softmax rowsum. Reciprocal
                    # via DMA-reshape to [128,4] so it runs 128 lanes wide.
                    cs65 = ep.tile([65, 512], f32, tag="rec65", bufs=3)
                    nc.vector.tensor_copy(cs65[:, :], cps[0:D + 1, :])
                    rsP = ep.tile([128, 4], f32, tag="rsP")
                    nc.sync.dma_start(out=rsP[:, :], in_=cs65[64:65, :])
                    rPr = ep.tile([128, 4], f32, tag="rPr")
                    nc.vector.reciprocal(rPr[:, :], rsP[:, :])
                    rec0 = ep.tile([1, 512], f32, tag="rec0")
                    nc.sync.dma_start(out=rec0[:, :], in_=rPr[:, :])
                    rb = ep.tile([64, 512], f32, tag="recb")
                    nc.gpsimd.partition_broadcast(rb[:, :], rec0[:, :])
                    nc.vector.tensor_mul(
                        ctxh[h][:, q0:q0 + 512], cs65[0:64, :], rb[:, :])
                    # slabs for BOTH same-rank slots (cross-group slot is a
                    # duplicate; keeps the SPMD stream core-independent).
                    for dup in range(2):
                        eng = nc.gpsimd if dup == 0 else nc.sync
                        r0 = OSL * (4 * dup + qs) + 64 * h
                        eng.dma_start(
                            out=a2a_in[r0:r0 + 64, :],
                            in_=ctxh[h][:, q0:q0 + 512])

            # ---------- phase A: head-pair-01 projections + rope.
            # q for all seq groups first, then k, then v: the first exp
            # needs only q+k of sg0-3, so the v bulk (which the DAG
            # scheduler would otherwise front-load onto the PE stream)
            # emits after the attention-critical work. ----------
            partsA = {sg: granules_for(sg, 512 * sg) for sg in range(4)}
            for sg in range(4):
                for f in partsA[sg][0]:            # q + rope
                    f()
            for sg in range(4):
                for f in partsA[sg][1]:            # k + rope
                    f()
            for sg in range(3):
                for f in partsA[sg][2]:            # v (sg3's v -> pass C0)
                    f()

            # tiny warmup collective: pre-arms ncfw so the real AllToAlls'
            # trigger-to-start latency is paid here, off the critical path
            warm_in = nc.dram_tensor("warm_in", [NC, 1, 64], bf16)
            warm_out = nc.dram_tensor("warm_out", [NC, 1, 64], bf16)
            nc.gpsimd.collective_compute(
                "AllToAll", mybir.AluOpType.bypass,
                replica_groups=[list(range(NC))],
                ins=[warm_in.ap().opt()],
                outs=[warm_out.ap().opt()])
            # full-size warmup: arms the 1MB transfer path so the real
            # a2a#A doesn't pay a cold start; runs during early phase C
            # when NeuronLink and HBM are otherwise idle.
            warm2_in = nc.dram_tensor("warm2_in", [NC * OSL, RB], bf16)
            warm2_out = nc.dram_tensor("warm2_out", [NC * OSL, RB], bf16)
            nc.gpsimd.collective_compute(
                "AllToAll", mybir.AluOpType.bypass,
                replica_groups=[list(range(NC))],
                ins=[warm2_in.ap().opt()],
                outs=[warm2_out.ap().opt()])

            # wo halves load during phase C (1MB each): issued after pass
            # C0 so the wire never contends with startup-critical HBM.
            woA = pp.tile([128, 4 * HID], bf16, tag="woLA", name="woA")
            woB = pp.tile([128, 4 * HID], bf16, tag="woLB", name="woB")

            def wo_load():
                # SCALAR engine ring: its DMA ring is empty during phase C,
                # so the 4MB wire never queues in front of the norm hop
                # DMAs (same-ring transfers execute in order). Costs two
                # ~0.8us pauses of the exp stream, once.
                nc.scalar.dma_start(out=woA[:, :], in_=woad[:, :])
                nc.scalar.dma_start(out=woB[:, :], in_=wobd[:, :])

            # ---------- phase C: head-pair-01 passes; head-pair-23
            # projections drain as micro-granules inside the passes ------
            _, _, vg3 = granules_for(3, 512 * 3)
            gparts = {sg: granules_for(sg, 512 * (sg % 4))
                      for sg in range(4, 8)}
            gqC = list(vg3)
            for sg in range(4, 8):
                gqC.extend(gparts[sg][1])          # k + rope
            gqC.append(wo_load)
            for sg in range(4, 8):
                gqC.extend(gparts[sg][2])          # v
            gqC.extend(gparts[4][0])               # q sg4 (pass D0)
            per_pass = (len(gqC) + 3) // 4
            for i in range(4):
                take = gqC[:per_pass]
                gqC = gqC[per_pass:]
                attn_pass(0, i, take)

            # a2a#A: head-pair-01 slabs; hides under phase D.
            nc.gpsimd.collective_compute(
                "AllToAll", mybir.AluOpType.bypass,
                replica_groups=[list(range(NC))],
                ins=[a2aA_in.ap().opt()],
                outs=[a2aA_out.ap().opt()])

            # ---------- phase D: head-pair-23 passes + A-half outproj ----
            from concourse.bass import ds as _ds

            def cx_load(cx, a2a_out):
                # in-group slabs only, selected with a runtime row offset
                # derived from this core's partition id (group base 4*(c//4)).
                for s2 in range(NG):
                    eng = nc.sync if s2 % 2 == 0 else nc.gpsimd
                    pid = eng.partition_id()
                    row = ((pid // NG) * NG + s2) * OSL
                    eng.dma_start(out=cx[s2][:, :],
                                  in_=a2a_out[_ds(row, OSL), :])

            def cxA_load():
                cx_load(cxA, a2aA_out)

            gqD = []
            for sg in (5, 6, 7):
                gqD.extend(gparts[sg][0])          # q sg5-7
            attn_pass(1, 0, gqD[:10])              # q sg5, sg6
            attn_pass(1, 1, gqD[10:])              # q sg7
            # cxA loads mid-D2: by then a2a#A is long done, so the waiting
            # DMAs never head-of-line-block the norm queues.
            attn_pass(1, 2, [None] * 16 + [cxA_load])
            attn_pass(1, 3)

            # a2a#B: head-pair-23 slabs; the serial tail.
            nc.gpsimd.collective_compute(
                "AllToAll", mybir.AluOpType.bypass,
                replica_groups=[list(range(NC))],
                ins=[a2aB_in.ap().opt()],
                outs=[a2aB_out.ap().opt()])

            _cmB.__exit__(None, None, None)
            _cmA.__exit__(None, None, None)
            _cmO = tc.tile_pool(name="psO", bufs=1, space="PSUM")
            psO = _cmO.__enter__()

            # 8 persistent accumulators. The A-half outproj (deps all ready
            # at D3 end) runs INSIDE the a2a#B wait and doubles as PE
            # keep-warm; its banks stay open until the B-half matmuls
            # continue the accumulation once the exchange lands. Keep-warm
            # dummies before it accumulate garbage into ops[0]; the A-half
            # s==0 matmul's start flag resets the bank.
            ops = [psO.tile([128, 512], f32, tag=f"ops{ot}", name=f"ops{ot}")
                   for ot in range(8)]
            dumsrc = pp.tile([128, 512], bf16, tag="dumsrc")
            nc.gpsimd.memset(dumsrc[:, :], 0.0)
            nc.vector.tensor_copy(
                dumsrc[0:64, :], ctxh[1][:, SB - 512:SB])
            for i in range(N_DUMMY):
                nc.tensor.matmul(
                    ops[0][:, :], lhsT=woB[:, 0:128], rhs=dumsrc[:, :],
                    start=True, stop=True)
            for s in range(NG):
                for ot in range(8):
                    nc.tensor.matmul(
                        ops[ot][:, :],
                        lhsT=woA[:, 1024 * s + 128 * ot:
                                 1024 * s + 128 * (ot + 1)],
                        rhs=cxA[s][:, :],
                        start=(s == 0), stop=False)

            cx_load(cxB, a2aB_out)
            for s in range(NG):
                for ot in range(8):
                    nc.tensor.matmul(
                        ops[ot][:, :],
                        lhsT=woB[:, 1024 * s + 128 * ot:
                                 1024 * s + 128 * (ot + 1)],
                        rhs=cxB[s][:, :],
                        start=False, stop=(s == NG - 1))
                if s == NG - 1:
                    for ot in range(8):
                        osb = ep.tile([128, RB], bf16, tag="osb", bufs=3)
                        nc.vector.tensor_scalar(
                            osb[:, :], ops[ot][:, :], 1.0,
                            bo_sb[:, ot:ot + 1],
                            mybir.AluOpType.mult, mybir.AluOpType.add)
                        eng = nc.sync if ot % 2 == 0 else nc.gpsimd
                        eng.dma_start(
                            out=out_ext[128 * ot:128 * (ot + 1), :],
                            in_=osb[:, :])
            _cmO.__exit__(None, None, None)

    nc.finalize()
    return nc


def _host_tables():
    inv = 1.0 / (ROPE_BASE ** (np.arange(0, D, 2, dtype=np.float64) / D))
    pos = np.arange(S, dtype=np.float64)
    freqs = np.outer(pos, inv)                      # [S, 32]
    emb = np.concatenate([freqs, freqs], axis=-1)   # [S, 64]
    cosT = np.cos(emb).T.astype(np.float32)         # [64, S]
    sinT = np.sin(emb).T.astype(np.float32)
    sinS = np.concatenate([-sinT[:32], sinT[32:]], axis=0)
    cos2 = np.ascontiguousarray(np.tile(cosT, (2, 1)))   # [128, S]
    sin2 = np.ascontiguousarray(np.tile(sinS, (2, 1)))
    return cos2, sin2


def kernel(**inputs):
    import ml_dtypes
    from concourse.bass_utils import run_bass_kernel_spmd

    global _cached, _last_in_maps
    if _cached is None:
        _cached = _build_nc()
    nc = _cached

    bf = ml_dtypes.bfloat16
    hs = np.asarray(inputs["hidden_states"], dtype=np.float32)
    Wq = np.asarray(inputs["Wq"], dtype=np.float32)
    bq = np.asarray(inputs["bq"], dtype=np.float32)
    Wk = np.asarray(inputs["Wk"], dtype=np.float32)
    bk = np.asarray(inputs["bk"], dtype=np.float32)
    Wv = np.asarray(inputs["Wv"], dtype=np.float32)
    bv = np.asarray(inputs["bv"], dtype=np.float32)
    Wo = np.asarray(inputs["Wo"], dtype=np.float32)
    bo = np.asarray(inputs["bo"], dtype=np.float32)

    cos2, sin2 = _host_tables()
    cs = np.ascontiguousarray(
        np.concatenate([cos2, sin2], axis=1)).astype(bf)   # [128, 2S]
    bo2 = bo + bv @ Wo.T                                 # fold v-bias exactly
    bo2m = np.ascontiguousarray(bo2.reshape(8, 128).T)   # [128, 8]
    pidx = np.arange(128)
    pm = np.where(pidx % 64 < 32, pidx + 32, pidx - 32)
    permM = np.zeros((128, 128), dtype=np.float32)
    permM[pm, pidx] = 1.0                                # [k, m]: k==perm(m)
    permM = permM.astype(bf)

    xTb = [np.ascontiguousarray(hs[g].T).astype(bf) for g in range(2)]

    in_maps = []
    for c in range(NC):
        g, r = divmod(c, NG)
        # wqkv: t-major, then head-pair, then 128-col chunk.
        wq6 = np.empty((128, 6 * 1024), dtype=np.float32)
        for t, W in enumerate((Wq, Wk, Wv)):
            for p in range(2):
                rows = slice(256 * r + 128 * p, 256 * r + 128 * (p + 1))
                wt = W[rows, :].T.reshape(8, 128, 128)      # [c, hid, out]
                wq6[:, 2048 * t + 1024 * p:2048 * t + 1024 * (p + 1)] = (
                    wt.transpose(1, 0, 2).reshape(128, 1024))
        # bqk cols: [q-p0, q-p1, k-p0, k-p1]
        bqk4 = np.empty((128, 4), dtype=np.float32)
        for t, b in enumerate((bq * 0.125, bk)):
            for p in range(2):
                rows = slice(256 * r + 128 * p, 256 * r + 128 * (p + 1))
                bqk4[:, 2 * t + p] = b[rows]
        # wo halves: block s' = Wo cols for in-group src s''s head pair
        # (A=01, B=23); identical on every core since the hid-chunk of
        # in-group src s' depends only on s' (its within-group rank).
        woLA = np.empty((128, 4 * 1024), dtype=np.float32)
        woLB = np.empty((128, 4 * 1024), dtype=np.float32)
        for s in range(NG):
            base = 256 * s
            woLA[:, 1024 * s:1024 * (s + 1)] = Wo[:, base:base + 128].T
            woLB[:, 1024 * s:1024 * (s + 1)] = (
                Wo[:, base + 128:base + 256].T)
        in_maps.append({
            "xT": xTb[g],
            "wqkv": np.ascontiguousarray(wq6).astype(bf),
            "woLA": np.ascontiguousarray(woLA).astype(bf),
            "woLB": np.ascontiguousarray(woLB).astype(bf),
            "bqk": np.ascontiguousarray(bqk4),
            "bo2": bo2m,
            "cs": cs,
            "perm": permM,
        })

    _last_in_maps = in_maps
    res = run_bass_kernel_spmd(nc, in_maps, core_ids=list(range(NC)))
    out = np.empty((2, S, HID), dtype=np.float32)
    for c in range(NC):
        g, r = divmod(c, NG)
        out[g, RB * r:RB * (r + 1), :] = res.results[c]["out"].T.astype(np.float32)
    return out
